# revision 15
# baseline (speedup 1.0000x reference)
"""CliffordLinearSimple on 8 Trainium2 NeuronCores.

Math (per reference):
    sv   = x[:, :, SV_IDX]                      # [B, IN_F, 9]  (scalar+vector slots)
    svo  = sv.reshape(B, IN_F*9) @ W.T + b      # [B, OUT_F*9]
    v    = svo.reshape(B, OUT_F, 9)[:, :, 1:]   # [B, OUT_F, 8]
    biv  = v[:, :, IU] * v[:, :, JU]            # [B, OUT_F, 28]
    out[..., SV_IDX] = svo; out[..., BIV_IDX] = biv; rest 0

Distribution: tensor-parallel over OUT_F (row-split W): core c owns out
features [c*128, (c+1)*128).  Every core gets the full sv (gathered and
transposed on host -- only 9/256 of x's last dim is ever read), its W row
shard (pre-transposed to the PE's [K, N] layout and cast to bf16 on host),
and its bias shard.  Each core computes its [256, 128, 37] compact output
(9 linear slots + 28 bivector products); the host scatters the compact
slots into the zero-filled [256, 1024, 256] multivector output.

Matmul runs in bf16 (PE 1 cycle/row vs 4 for fp32; halves W HBM traffic);
PSUM accumulation is fp32.  Bias is added via an extra K=1 matmul with a
ones vector.  Bivector products are exact fp32 on DVE, reading PSUM
directly per 32-out-feature chunk (N_TILE=288 keeps PSUM tiles aligned to
out-feature boundaries), grouped by index distance d=j-i so the 28 pair
products collapse into 7 strided tensor_mul ops.

DMA plumbing: the two HWDGE rings (sync + scalar) each carry half of the
W stream with svT interleaved (a single ring serializes transfers and
delays the first matmul); compact outputs leave via SWDGE (gpsimd).
"""
import sys

if "/opt/trn_rl_repo" not in sys.path:
    sys.path.insert(0, "/opt/trn_rl_repo")

from contextlib import ExitStack

import ml_dtypes
import numpy as np

import concourse.bass as bass
import concourse.tile as tile
from concourse import bacc, mybir
from concourse.bass_utils import run_bass_kernel_spmd

ALG_DIM = 8
D1 = 9
MV_DIM = 256
B, IN_F, OUT_F = 256, 1024, 1024
POW2 = np.array([2 ** i for i in range(ALG_DIM)])
SV_IDX = np.concatenate([[0], POW2])
IU, JU = np.triu_indices(ALG_DIM, 1)
BIV_IDX = POW2[IU] + POW2[JU]
NPAIR = len(IU)  # 28
NSLOT = 1 + ALG_DIM + NPAIR  # 37
NCORES = 8
OF = OUT_F // NCORES  # 128 out features per core

# bivector pairs grouped by distance d = j - i: one strided mul per group
PAIRS_BY_D = [(d, [(i, i + d) for i in range(ALG_DIM - d)]) for d in range(1, ALG_DIM)]
# compact-slot order: 9 sv slots, then the d-grouped pair products
IDX37 = list(SV_IDX) + [POW2[i] + POW2[j] for d, prs in PAIRS_BY_D for (i, j) in prs]
IDX37 = np.array(IDX37)
assert len(set(IDX37.tolist())) == NSLOT

# full-size tiling: K = IN_F*9 = 9216 = KT*128; N per core = OF*9 = 1152.
# NTILES: out-feature-aligned (mult of 9) PSUM tile widths (<=512 f32/bank);
# last (smallest) tile processed last to minimize the kernel tail.
# KTLS: k-group sizes (in 128-deep k-tiles); small leading groups get the
# first W/svT blocks on-chip quickly so real matmuls start early.
FULL_CFG = dict(KT=72, KTLS=(9, 9, 18, 18, 18), OF=128, NTILES=(432, 432, 288), BT=2, WARM=14)


def build_core_program(KT, KTLS, OF, NTILES, BT, WARM=0):
    """SPMD per-core program: C[128*BT, OF*9] = svT.T @ Wh + b, then the
    9-slot copy + 28 bivector products into the compact output."""
    assert KT == sum(KTLS) and sum(NTILES) == OF * D1
    assert all(nt % D1 == 0 and nt <= 512 for nt in NTILES)
    NT = len(NTILES)
    NOFF = [sum(NTILES[:i]) for i in range(NT)]  # column offsets
    KOFF = [sum(KTLS[:i]) for i in range(len(KTLS))]  # k-group offsets
    Bfull = BT * 128
    f32, bf16 = mybir.dt.float32, mybir.dt.bfloat16

    nc = bacc.Bacc("TRN2", target_bir_lowering=False, debug=False)
    svT_d = nc.dram_tensor("svT", [128, KT, Bfull], bf16, kind="ExternalInput").ap()
    # flat per-n W: k-group blocks [128, ktl, NTILE] packed contiguously in
    # group order, so every DMA reads one fully-sequential DRAM region
    W_ds = [
        nc.dram_tensor(f"Wh{n}", [KT * 128 * NTILES[n]], bf16, kind="ExternalInput").ap()
        for n in range(NT)
    ]
    b_d = nc.dram_tensor("bh", [1, OF * D1], bf16, kind="ExternalInput").ap()
    out_d = nc.dram_tensor("outc", [Bfull, OF, NSLOT], f32, kind="ExternalOutput").ap()

    rings = [nc.sync, nc.scalar]  # the two HWDGE rings

    with tile.TileContext(nc) as tc:
        with ExitStack() as ctx:
            const = ctx.enter_context(tc.tile_pool(name="const", bufs=1))
            wpool = ctx.enter_context(tc.tile_pool(name="wpool", bufs=6))
            spool = ctx.enter_context(tc.tile_pool(name="spool", bufs=3))
            pspool = ctx.enter_context(
                tc.tile_pool(name="pspool", bufs=NT * BT, space="PSUM")
            )

            # svT and b ride the otherwise-idle SWDGE queues (gpsimd) so the
            # two HWDGE rings carry nothing but the W stream
            svT = const.tile([128, KT, Bfull], bf16)
            b_sb = const.tile([1, OF * D1], bf16)
            nc.gpsimd.dma_start(b_sb[:], b_d)
            ones = const.tile([1, 128], bf16)
            nc.vector.memset(ones[:], 1.0)

            # all PSUM accumulators live for the whole kernel (NT*BT banks)
            ps = {
                (m, n): pspool.tile([128, NTILES[n]], f32, name=f"ps{m}_{n}", tag="ps")
                for n in range(NT)
                for m in range(BT)
            }

            # PE warm-up with no DMA deps: junk matmuls into ps[0,0] (its
            # bias matmul below re-opens the bank with start=True), so the
            # HAM clock gate is already released when real work arrives.
            if WARM:
                warm_rhs = const.tile([1, NTILES[0]], bf16)
                nc.vector.memset(warm_rhs[:], 0.0)
                for _ in range(WARM):
                    nc.tensor.matmul(
                        ps[(0, 0)][:], ones[:], warm_rhs[:],
                        start=True, stop=True, skip_group_check=True,
                    )

            # bias first (start=True initializes each bank); the K-sweep
            # accumulates on top and these early matmuls double as warm-up
            for n in range(NT):
                for m in range(BT):
                    nc.tensor.matmul(
                        ps[(m, n)][:],
                        ones[:],
                        b_sb[:, NOFF[n]:NOFF[n] + NTILES[n]],
                        start=True,
                        stop=False,
                    )

            ring_i = 0

            def next_ring():
                nonlocal ring_i
                ring_i ^= 1
                return rings[ring_i]

            for n in range(NT):
                for g, ktl_n in enumerate(KTLS):
                    k0, k1 = KOFF[g], KOFF[g] + ktl_n
                    if n == 0:
                        # svT chunk g feeds exactly the g-th k-group
                        nc.gpsimd.dma_start(svT[:, k0:k1, :], svT_d[:, k0:k1, :])
                    wt = wpool.tile([128, ktl_n, NTILES[n]], bf16, name="wt", tag="wt")
                    blk = W_ds[n][k0 * 128 * NTILES[n]:k1 * 128 * NTILES[n]]
                    next_ring().dma_start(wt[:], blk.rearrange("(p r) -> p r", p=128))
                    for m in range(BT):
                        for ktl in range(ktl_n):
                            kt = k0 + ktl
                            nc.tensor.matmul(
                                ps[(m, n)][:],
                                svT[:, kt, m * 128:(m + 1) * 128],
                                wt[:, ktl],
                                start=False,
                                stop=(kt == KT - 1),
                            )
                for m in range(BT):
                    # scatter this chunk: one PSUM->SBUF copy of the 9 sv
                    # slots, then the pair products read the SBUF copy (DVE
                    # can read at most one PSUM operand per instruction).
                    # All on DVE: the scalar engine issues half the W-stream
                    # DMAs and compute there would bubble that ring.
                    och = NTILES[n] // D1
                    psr = ps[(m, n)].rearrange("p (o j) -> p o j", j=D1)
                    st = spool.tile([128, och, NSLOT], f32, name="st", tag="st")
                    nc.vector.tensor_copy(st[:, :, 0:D1], psr[:])
                    s = D1
                    for d, prs in PAIRS_BY_D:
                        w = len(prs)  # pairs (i, i+d), i = 0..w-1
                        nc.vector.tensor_mul(
                            st[:, :, s:s + w],
                            st[:, :, 1:1 + w],
                            st[:, :, 1 + d:1 + d + w],
                        )
                        s += w
                    nc.gpsimd.dma_start(
                        out_d[m * 128:(m + 1) * 128, NOFF[n] // D1:NOFF[n] // D1 + och, :],
                        st[:],
                    )

    nc.finalize()
    return nc


_PROGRAM = None


def _get_program():
    global _PROGRAM
    if _PROGRAM is None:
        _PROGRAM = build_core_program(**FULL_CFG)
    return _PROGRAM


def _prep_inputs(x, W, b):
    bf16 = ml_dtypes.bfloat16
    KT, NTILES = FULL_CFG["KT"], FULL_CFG["NTILES"]
    NOFF = [sum(NTILES[:i]) for i in range(len(NTILES))]
    # svT[p, kt, m] = sv[m, kt*128 + p], sv = x[:, :, SV_IDX] flattened
    sv = np.ascontiguousarray(x[:, :, SV_IDX]).reshape(B, IN_F * D1)
    svT = np.ascontiguousarray(sv.reshape(B, KT, 128).transpose(2, 1, 0)).astype(bf16)

    Wb = W.astype(bf16)
    # Wr[c, o', kt, p] with o' the core-local output column
    Wr = Wb.reshape(NCORES, OF * D1, KT, 128)
    KTLS = FULL_CFG["KTLS"]
    KOFF = [sum(KTLS[:i]) for i in range(len(KTLS))]
    in_maps = []
    for c in range(NCORES):
        m = {
            "svT": svT,
            "bh": np.ascontiguousarray(b[c * OF * D1:(c + 1) * OF * D1]).astype(bf16).reshape(1, OF * D1),
        }
        for n, nt in enumerate(NTILES):
            # per k-group block [p, ktl, jj] = W_core[NOFF[n]+jj, kt*128+p],
            # blocks raveled + concatenated (matches the device-side slices)
            sub = Wr[c, NOFF[n]:NOFF[n] + nt]  # [jj, kt, p]
            parts = []
            for g, ktl in enumerate(KTLS):
                blk = sub[:, KOFF[g]:KOFF[g] + ktl]  # [jj, ktl, p]
                parts.append(np.ascontiguousarray(blk.transpose(2, 1, 0)).ravel())
            m[f"Wh{n}"] = np.concatenate(parts)
        in_maps.append(m)
    return in_maps


def run(x, W, b, trace=False):
    x = np.asarray(x, dtype=np.float32)
    W = np.asarray(W, dtype=np.float32)
    b = np.asarray(b, dtype=np.float32)
    in_maps = _prep_inputs(x, W, b)
    nc = _get_program()
    res = run_bass_kernel_spmd(nc, in_maps, core_ids=list(range(NCORES)), trace=trace)
    comp = np.concatenate([res.results[c]["outc"] for c in range(NCORES)], axis=1)
    out = np.zeros((B, OUT_F, MV_DIM), dtype=np.float32)
    out[:, :, IDX37] = comp
    return out, res


def kernel(x, W, b):
    out, _ = run(x, W, b)
    return out


# revision 16
# speedup vs baseline: 1.0628x; 1.0628x over previous
"""CliffordLinearSimple on 8 Trainium2 NeuronCores.

Math (per reference):
    sv   = x[:, :, SV_IDX]                      # [B, IN_F, 9]  (scalar+vector slots)
    svo  = sv.reshape(B, IN_F*9) @ W.T + b      # [B, OUT_F*9]
    v    = svo.reshape(B, OUT_F, 9)[:, :, 1:]   # [B, OUT_F, 8]
    biv  = v[:, :, IU] * v[:, :, JU]            # [B, OUT_F, 28]
    out[..., SV_IDX] = svo; out[..., BIV_IDX] = biv; rest 0

Distribution: tensor-parallel over OUT_F (row-split W): core c owns out
features [c*128, (c+1)*128).  Every core gets the full sv (gathered and
transposed on host -- only 9/256 of x's last dim is ever read), its W row
shard (pre-transposed to the PE's [K, N] layout and cast to bf16 on host),
and its bias shard.  Each core computes its [256, 128, 37] compact output
(9 linear slots + 28 bivector products); the host scatters the compact
slots into the zero-filled [256, 1024, 256] multivector output.

Matmul runs in bf16 (PE 1 cycle/row vs 4 for fp32; halves W HBM traffic);
PSUM accumulation is fp32.  Bias is added via an extra K=1 matmul with a
ones vector.  Bivector products are exact fp32 on DVE, reading PSUM
directly per 32-out-feature chunk (N_TILE=288 keeps PSUM tiles aligned to
out-feature boundaries), grouped by index distance d=j-i so the 28 pair
products collapse into 7 strided tensor_mul ops.

DMA plumbing: the two HWDGE rings (sync + scalar) each carry half of the
W stream with svT interleaved (a single ring serializes transfers and
delays the first matmul); compact outputs leave via SWDGE (gpsimd).
"""
import sys

if "/opt/trn_rl_repo" not in sys.path:
    sys.path.insert(0, "/opt/trn_rl_repo")

from contextlib import ExitStack

import ml_dtypes
import numpy as np

import concourse.bass as bass
import concourse.tile as tile
from concourse import bacc, mybir
from concourse.bass_utils import run_bass_kernel_spmd

ALG_DIM = 8
D1 = 9
MV_DIM = 256
B, IN_F, OUT_F = 256, 1024, 1024
POW2 = np.array([2 ** i for i in range(ALG_DIM)])
SV_IDX = np.concatenate([[0], POW2])
IU, JU = np.triu_indices(ALG_DIM, 1)
BIV_IDX = POW2[IU] + POW2[JU]
NPAIR = len(IU)  # 28
NSLOT = 1 + ALG_DIM + NPAIR  # 37
NCORES = 8
OF = OUT_F // NCORES  # 128 out features per core

# bivector pairs grouped by distance d = j - i: one strided mul per group
PAIRS_BY_D = [(d, [(i, i + d) for i in range(ALG_DIM - d)]) for d in range(1, ALG_DIM)]
# compact-slot order: 9 sv slots, then the d-grouped pair products
IDX37 = list(SV_IDX) + [POW2[i] + POW2[j] for d, prs in PAIRS_BY_D for (i, j) in prs]
IDX37 = np.array(IDX37)
assert len(set(IDX37.tolist())) == NSLOT

# full-size tiling: K = IN_F*9 = 9216 = KT*128; N per core = OF*9 = 1152.
# NTILES: out-feature-aligned (mult of 9) PSUM tile widths (<=512 f32/bank);
# last (smallest) tile processed last to minimize the kernel tail.
# KTLS: k-group sizes (in 128-deep k-tiles); small leading groups get the
# first W/svT blocks on-chip quickly so real matmuls start early.
FULL_CFG = dict(KT=72, KTLS=(18, 18, 18, 18), OF=128, NTILES=(432, 432, 288), BT=2, WARM=16)


def build_core_program(KT, KTLS, OF, NTILES, BT, WARM=0):
    """SPMD per-core program: C[128*BT, OF*9] = svT.T @ Wh + b, then the
    9-slot copy + 28 bivector products into the compact output."""
    assert KT == sum(KTLS) and sum(NTILES) == OF * D1
    assert all(nt % D1 == 0 and nt <= 512 for nt in NTILES)
    NT = len(NTILES)
    NOFF = [sum(NTILES[:i]) for i in range(NT)]  # column offsets
    KOFF = [sum(KTLS[:i]) for i in range(len(KTLS))]  # k-group offsets
    Bfull = BT * 128
    f32, bf16 = mybir.dt.float32, mybir.dt.bfloat16

    nc = bacc.Bacc("TRN2", target_bir_lowering=False, debug=False)
    svT_d = nc.dram_tensor("svT", [128, KT, Bfull], bf16, kind="ExternalInput").ap()
    # flat per-n W: each k-group block is stored as two contiguous
    # half-blocks [128, ktl/2, NTILE]; the halves stream down the two HWDGE
    # rings in parallel, so every block arrives at combined-ring bandwidth
    # in exact consumption order
    W_ds = [
        nc.dram_tensor(f"Wh{n}", [KT * 128 * NTILES[n]], bf16, kind="ExternalInput").ap()
        for n in range(NT)
    ]
    b_d = nc.dram_tensor("bh", [1, OF * D1], bf16, kind="ExternalInput").ap()
    out_d = nc.dram_tensor("outc", [Bfull, OF, NSLOT], f32, kind="ExternalOutput").ap()

    rings = [nc.sync, nc.scalar]  # the two HWDGE rings

    with tile.TileContext(nc) as tc:
        with ExitStack() as ctx:
            const = ctx.enter_context(tc.tile_pool(name="const", bufs=1))
            wpool = ctx.enter_context(tc.tile_pool(name="wpool", bufs=6))
            spool = ctx.enter_context(tc.tile_pool(name="spool", bufs=3))
            pspool = ctx.enter_context(
                tc.tile_pool(name="pspool", bufs=NT * BT, space="PSUM")
            )

            svT = const.tile([128, KT, Bfull], bf16)
            b_sb = const.tile([1, OF * D1], bf16)
            rings[1].dma_start(b_sb[:], b_d)
            ones = const.tile([1, 128], bf16)
            nc.vector.memset(ones[:], 1.0)

            # all PSUM accumulators live for the whole kernel (NT*BT banks)
            ps = {
                (m, n): pspool.tile([128, NTILES[n]], f32, name=f"ps{m}_{n}", tag="ps")
                for n in range(NT)
                for m in range(BT)
            }

            # PE warm-up with no DMA deps: junk matmuls into ps[0,0] (its
            # bias matmul below re-opens the bank with start=True), so the
            # HAM clock gate is already released when real work arrives.
            if WARM:
                warm_rhs = const.tile([1, NTILES[0]], bf16)
                nc.vector.memset(warm_rhs[:], 0.0)
                for _ in range(WARM):
                    nc.tensor.matmul(
                        ps[(0, 0)][:], ones[:], warm_rhs[:],
                        start=True, stop=True, skip_group_check=True,
                    )

            # bias first (start=True initializes each bank); the K-sweep
            # accumulates on top and these early matmuls double as warm-up
            for n in range(NT):
                for m in range(BT):
                    nc.tensor.matmul(
                        ps[(m, n)][:],
                        ones[:],
                        b_sb[:, NOFF[n]:NOFF[n] + NTILES[n]],
                        start=True,
                        stop=False,
                    )

            for n in range(NT):
                for g, ktl_n in enumerate(KTLS):
                    k0, k1 = KOFF[g], KOFF[g] + ktl_n
                    kh = ktl_n // 2
                    if n == 0:
                        # svT chunk g feeds exactly the g-th k-group
                        rings[0].dma_start(svT[:, k0:k0 + kh, :], svT_d[:, k0:k0 + kh, :])
                        rings[1].dma_start(svT[:, k0 + kh:k1, :], svT_d[:, k0 + kh:k1, :])
                    wt = wpool.tile([128, ktl_n, NTILES[n]], bf16, name="wt", tag="wt")
                    half = kh * 128 * NTILES[n]
                    base = k0 * 128 * NTILES[n]
                    for h in range(2):
                        blk = W_ds[n][base + h * half:base + (h + 1) * half]
                        rings[h].dma_start(
                            wt[:, h * kh:(h + 1) * kh, :],
                            blk.rearrange("(p r) -> p r", p=128),
                        )
                    for m in range(BT):
                        for ktl in range(ktl_n):
                            kt = k0 + ktl
                            nc.tensor.matmul(
                                ps[(m, n)][:],
                                svT[:, kt, m * 128:(m + 1) * 128],
                                wt[:, ktl],
                                start=False,
                                stop=(kt == KT - 1),
                            )
                for m in range(BT):
                    # scatter this chunk: one PSUM->SBUF copy of the 9 sv
                    # slots, then the pair products read the SBUF copy (DVE
                    # can read at most one PSUM operand per instruction).
                    # All on DVE: the scalar engine issues half the W-stream
                    # DMAs and compute there would bubble that ring.
                    och = NTILES[n] // D1
                    psr = ps[(m, n)].rearrange("p (o j) -> p o j", j=D1)
                    st = spool.tile([128, och, NSLOT], f32, name="st", tag="st")
                    nc.vector.tensor_copy(st[:, :, 0:D1], psr[:])
                    s = D1
                    for d, prs in PAIRS_BY_D:
                        w = len(prs)  # pairs (i, i+d), i = 0..w-1
                        nc.vector.tensor_mul(
                            st[:, :, s:s + w],
                            st[:, :, 1:1 + w],
                            st[:, :, 1 + d:1 + d + w],
                        )
                        s += w
                    nc.gpsimd.dma_start(
                        out_d[m * 128:(m + 1) * 128, NOFF[n] // D1:NOFF[n] // D1 + och, :],
                        st[:],
                    )

    nc.finalize()
    return nc


_PROGRAM = None


def _get_program():
    global _PROGRAM
    if _PROGRAM is None:
        _PROGRAM = build_core_program(**FULL_CFG)
    return _PROGRAM


def _prep_inputs(x, W, b):
    bf16 = ml_dtypes.bfloat16
    KT, NTILES = FULL_CFG["KT"], FULL_CFG["NTILES"]
    NOFF = [sum(NTILES[:i]) for i in range(len(NTILES))]
    # svT[p, kt, m] = sv[m, kt*128 + p], sv = x[:, :, SV_IDX] flattened
    sv = np.ascontiguousarray(x[:, :, SV_IDX]).reshape(B, IN_F * D1)
    svT = np.ascontiguousarray(sv.reshape(B, KT, 128).transpose(2, 1, 0)).astype(bf16)

    Wb = W.astype(bf16)
    # Wr[c, o', kt, p] with o' the core-local output column
    Wr = Wb.reshape(NCORES, OF * D1, KT, 128)
    KTLS = FULL_CFG["KTLS"]
    KOFF = [sum(KTLS[:i]) for i in range(len(KTLS))]
    in_maps = []
    for c in range(NCORES):
        m = {
            "svT": svT,
            "bh": np.ascontiguousarray(b[c * OF * D1:(c + 1) * OF * D1]).astype(bf16).reshape(1, OF * D1),
        }
        for n, nt in enumerate(NTILES):
            # per k-group half-blocks [p, ktl/2, jj] = W_core[NOFF[n]+jj, kt*128+p],
            # raveled + concatenated (matches the device-side ring slices)
            sub = Wr[c, NOFF[n]:NOFF[n] + nt]  # [jj, kt, p]
            parts = []
            for g, ktl in enumerate(KTLS):
                kh = ktl // 2
                for h in range(2):
                    a = KOFF[g] + h * kh
                    blk = sub[:, a:a + kh]  # [jj, kh, p]
                    parts.append(np.ascontiguousarray(blk.transpose(2, 1, 0)).ravel())
            m[f"Wh{n}"] = np.concatenate(parts)
        in_maps.append(m)
    return in_maps


def run(x, W, b, trace=False):
    x = np.asarray(x, dtype=np.float32)
    W = np.asarray(W, dtype=np.float32)
    b = np.asarray(b, dtype=np.float32)
    in_maps = _prep_inputs(x, W, b)
    nc = _get_program()
    res = run_bass_kernel_spmd(nc, in_maps, core_ids=list(range(NCORES)), trace=trace)
    comp = np.concatenate([res.results[c]["outc"] for c in range(NCORES)], axis=1)
    out = np.zeros((B, OUT_F, MV_DIM), dtype=np.float32)
    out[:, :, IDX37] = comp
    return out, res


def kernel(x, W, b):
    out, _ = run(x, W, b)
    return out
